# revision 1
# baseline (speedup 1.0000x reference)
"""Trainium2 Bass kernel for nn_MultiHeadAttention_55894704390646.

Multi-head causal attention, B=2, S=2048, E=1024, H=16 heads, D=64.
Sharding: data-parallel over batch (2 groups) x tensor-parallel over heads
(4 heads per core). Each core computes a partial output-projection result
(row-split Wo); the host sums the 4 partials per batch and adds the bias.

Device-side dataflow (per core, all fp32 with float32r matmuls):
  - host supplies x^T [E, S] and pre-transposed weight slices, so every
    matmul contraction dim lands on SBUF partitions with no on-device
    transposes.
  - qT/kT computed in [d, tokens] layout, v in [tokens, d] layout.
  - scores computed transposed ([keys, queries]); softmax uses
    exp(s/8) with a multiplicative causal mask (no max subtraction needed:
    |s/8| is bounded by ~±6) and the denominator comes from a ones-column
    appended to v (M=65 matmul).
  - normalization multiplies by 1/denom broadcast across partitions via a
    PE outer product.
"""

import sys

if "/opt/trn_rl_repo" not in sys.path:
    sys.path.insert(0, "/opt/trn_rl_repo")

import numpy as np

import concourse.bass as bass
from concourse import bacc
import concourse.mybir as mybir
import concourse.tile as tile
from concourse.bass_utils import run_bass_kernel_spmd

B, S, E, H, D = 2, 2048, 1024, 16, 64
N_CORES = 8
DP = 2                 # batch groups
TP = 4                 # cores per batch group
HL = H // TP           # local heads per core = 4
DL = HL * D            # local head dims = 256
P = 128
NTB = S // P           # token blocks = 16
QC = 512               # query chunk
NQC = S // QC          # query chunks = 4
NKB_PER_QC = QC // P   # k-blocks per q chunk = 4
NPAIR = HL // 2        # head pairs = 2
NEO = E // QC          # output feature chunks of 512 = 2
NKO = E // P           # contraction blocks over E = 8

f32 = mybir.dt.float32
f32r = mybir.dt.float32r
EXP = mybir.ActivationFunctionType.Exp

_NC_CACHE = None


def _build_nc():
    nc = bacc.Bacc("TRN2", target_bir_lowering=False, debug=False)

    xT = nc.dram_tensor("xT", (E, S), f32r, kind="ExternalInput")
    wqT = nc.dram_tensor("wqT", (E, DL), f32r, kind="ExternalInput")
    wkT = nc.dram_tensor("wkT", (E, DL), f32r, kind="ExternalInput")
    wvT = nc.dram_tensor("wvT", (E, DL), f32r, kind="ExternalInput")
    woT = nc.dram_tensor("woT", (DL, E), f32r, kind="ExternalInput")
    out = nc.dram_tensor("out", (S, E), f32, kind="ExternalOutput")

    with tile.TileContext(nc) as tc:
        with (
            nc.allow_low_precision(reason="float32r is the intended matmul dtype"),
            tc.tile_pool(name="big", bufs=1) as big,
            tc.tile_pool(name="work", bufs=4) as work,
            tc.tile_pool(name="work2", bufs=2) as work2,
            tc.tile_pool(name="ps", bufs=2, space="PSUM") as ps,
            tc.tile_pool(name="ps_s", bufs=2, space="PSUM") as ps_s,
            tc.tile_pool(name="ps_ctx", bufs=2, space="PSUM") as ps_ctx,
        ):
            # ---- loads: q/k weights, then x^T per k-block, then v/o weights.
            # All persistent tensors are chunk-granular tiles so the Tile
            # scheduler sees fine-grained deps and can overlap phases.
            wqT_sb = big.tile([P, NKO, DL], f32r, tag="wqT")
            nc.sync.dma_start(wqT_sb[:], wqT[:].rearrange("(ko p) d -> p ko d", p=P))
            wkT_sb = big.tile([P, NKO, DL], f32r, tag="wkT")
            nc.sync.dma_start(wkT_sb[:], wkT[:].rearrange("(ko p) d -> p ko d", p=P))
            # x^T is loaded as 32 (ko, token-chunk) tiles so early projection
            # groups complete before the whole 8.4MB input lands.
            xT_r = xT[:].rearrange("(ko p) (c s) -> p ko c s", p=P, c=NQC)
            xT_q = [
                [
                    big.tile([P, QC], f32r, tag=f"xTq{ko}_{c}", name=f"xTq{ko}_{c}")
                    for c in range(NQC)
                ]
                for ko in range(NKO)
            ]
            for c in range(NQC):
                for ko in range(NKO):
                    nc.sync.dma_start(xT_q[ko][c][:], xT_r[:, ko, c, :])
                if c == 0:
                    wvT_sb = big.tile([P, NKO, DL], f32r, tag="wvT")
                    nc.sync.dma_start(
                        wvT_sb[:], wvT[:].rearrange("(ko p) d -> p ko d", p=P)
                    )
            woT_sb = big.tile([P, NPAIR, E], f32r, tag="woT")
            nc.sync.dma_start(woT_sb[:], woT[:].rearrange("(pr p) e -> p pr e", p=P))

            # causal masks for the 4 diagonal-region k-blocks of a q-chunk:
            # mask_i[k, q] = 1 if (k + 128*i) <= q else 0
            masks = []
            for i in range(NKB_PER_QC):
                m = big.tile([P, QC], mybir.dt.bfloat16, tag=f"mask{i}", name=f"mask{i}")
                nc.gpsimd.memset(m[:], 1.0)
                nc.gpsimd.affine_select(
                    out=m[:],
                    in_=m[:],
                    compare_op=mybir.AluOpType.is_ge,
                    fill=0.0,
                    base=-P * i,
                    pattern=[[1, QC]],
                    channel_multiplier=-1,
                )
                masks.append(m)

            ones_stage = big.tile([P, HL], f32, tag="ones_stage")
            nc.gpsimd.memset(ones_stage[:], 1.0)

            # chunk-granular persistent activation buffers
            qT_c = [[None] * NQC for _ in range(NPAIR)]
            kT_c = [[None] * NQC for _ in range(NPAIR)]
            for pr in range(NPAIR):
                for ch in range(NQC):
                    qT_c[pr][ch] = big.tile(
                        [P, QC], f32r, tag=f"qT{pr}{ch}", name=f"qT{pr}{ch}"
                    )
                    kT_c[pr][ch] = big.tile(
                        [P, QC], f32r, tag=f"kT{pr}{ch}", name=f"kT{pr}{ch}"
                    )
            v_tb = []
            for tb in range(NTB):
                vt = big.tile([P, HL, D + 1], f32r, tag=f"v{tb}", name=f"v{tb}")
                nc.vector.tensor_copy(vt[:, :, D], ones_stage[:, :])
                v_tb.append(vt)
            ctx_J = []
            for J in range(NQC):
                ctx_J.append(
                    big.tile([P, NPAIR, QC], f32r, tag=f"ctxT{J}", name=f"ctxT{J}")
                )

            def emit_qk_ch(ch):
                """qT/kT projections for one token chunk, all pairs."""
                for pr in range(NPAIR):
                    for wt_sb, dst in ((wqT_sb, qT_c), (wkT_sb, kT_c)):
                        pp = ps.tile([P, QC], f32, tag="mm", name=f"pp_{pr}_{ch}")
                        for ko in range(NKO):
                            nc.tensor.matmul(
                                pp[:],
                                wt_sb[:, ko, pr * P : (pr + 1) * P],
                                xT_q[ko][ch][:],
                                start=(ko == 0),
                                stop=(ko == NKO - 1),
                            )
                        nc.scalar.copy(dst[pr][ch][:], pp[:])

            def emit_v(tb0, tb1):
                for tb in range(tb0, tb1):
                    pv_full = ps.tile([P, QC], f32, tag="mm", name="pv_full")
                    pv = pv_full[:, 0:DL]
                    for ko in range(NKO):
                        nc.tensor.matmul(
                            pv[:],
                            xT_q[ko][tb // NKB_PER_QC][
                                :, (tb % NKB_PER_QC) * P : (tb % NKB_PER_QC + 1) * P
                            ],
                            wvT_sb[:, ko, :],
                            start=(ko == 0),
                            stop=(ko == NKO - 1),
                        )
                    nc.scalar.copy(
                        v_tb[tb][:, :, 0:D],
                        pv[:].rearrange("p (h d) -> p h d", h=HL),
                    )

            def normalize(ctx_ps, pr, r, J):
                """ctxT[h] = ctx[:64] / ctx[64] into its pair slot.

                The PSUM accumulator is evacuated to SBUF immediately (one ACT
                copy) so the bank frees for the next chunk; the reciprocal /
                broadcast / multiply chain then runs off the critical path."""
                cu = work2.tile([D, QC], f32, tag="cu")
                nc.scalar.copy(cu[:], ctx_ps[0:D, :])
                dn = work2.tile([1, QC], f32, tag="nrm", name="dn")
                nc.scalar.copy(dn[:], ctx_ps[D : D + 1, :])
                recip = work2.tile([1, QC], f32, tag="nrm", name="recip")
                nc.vector.reciprocal_approx_fast(recip[:], dn[:])
                dnb = work2.tile([64, QC], f32, tag="dnb")
                nc.gpsimd.partition_broadcast(dnb[:], recip[:])
                if r == 0:
                    nc.vector.tensor_tensor(
                        ctx_J[J][0:64, pr, :],
                        cu[:],
                        dnb[:],
                        mybir.AluOpType.mult,
                    )
                else:
                    tmp = work2.tile([64, QC], f32r, tag="ctmp")
                    nc.vector.tensor_tensor(
                        tmp[:], cu[:], dnb[:], mybir.AluOpType.mult
                    )
                    nc.sync.dma_start(ctx_J[J][64:128, pr, :], tmp[:])

            def attn_scores_group(pr, J, I):
                """Scores + exp + mask for k-block I of pair pr, chunk J."""
                ik = slice((I % NKB_PER_QC) * P, (I % NKB_PER_QC + 1) * P)
                kch = I // NKB_PER_QC
                s = ps_s.tile([P, 2, QC], f32, tag="s", name="s")
                nc.tensor.matmul(
                    s[:, 0, :],
                    kT_c[pr][kch][0:64, ik],
                    qT_c[pr][J][0:64, :],
                    start=True,
                    stop=True,
                )
                nc.tensor.matmul(
                    s[:, 1, :],
                    kT_c[pr][kch][64:128, ik],
                    qT_c[pr][J][64:128, :],
                    start=True,
                    stop=True,
                )
                pT = work.tile([P, 2, QC], f32r, tag="pT", name="pT")
                nc.scalar.activation(pT[:], s[:], EXP, scale=0.125)
                di = I - NKB_PER_QC * J
                if di >= 0:
                    nc.vector.tensor_tensor(
                        pT[:],
                        pT[:],
                        masks[di][:, None, :].to_broadcast((P, 2, QC)),
                        mybir.AluOpType.mult,
                    )
                return pT

            def emit_attn_pair(pr, J):
                """Attention for the head pair (2pr, 2pr+1) on query chunk J.
                The two heads' K=64 score matmuls go back-to-back into the two
                halves of one 2-bank PSUM tile with row groups 0/64, so the PE
                array runs them concurrently. AVs are skewed one k-block behind
                the scores so the in-order PE queue never waits on exp/mask."""
                h0, h1 = 2 * pr, 2 * pr + 1
                nI = NKB_PER_QC * (J + 1)
                ctx0 = ps_ctx.tile([D + 1, QC], f32, tag="ctx", name="ctx0")
                ctx1 = ps_ctx.tile([D + 1, QC], f32, tag="ctx", name="ctx1")

                def emit_av(I, pT):
                    nc.tensor.matmul(
                        ctx0[:], v_tb[I][:, h0, :], pT[:, 0, :],
                        start=(I == 0), stop=(I == nI - 1),
                    )
                    nc.tensor.matmul(
                        ctx1[:], v_tb[I][:, h1, :], pT[:, 1, :],
                        start=(I == 0), stop=(I == nI - 1),
                    )

                prev_pT = pending.pop() if pending else attn_scores_group(pr, J, 0)
                for I in range(1, nI):
                    pT = attn_scores_group(pr, J, I)
                    emit_av(I - 1, prev_pT)
                    prev_pT = pT
                # prefetch the NEXT chunk's first scores group before the last
                # AV + normalize so the PE queue never drains at chunk starts
                nxt = chain.pop(0) if chain else None
                if nxt is not None:
                    pending.append(attn_scores_group(nxt[0], nxt[1], 0))
                emit_av(nI - 1, prev_pT)
                # r=1 head first: its ctx reaches ctx_J via an SBUF shift DMA,
                # so keep that latency off the critical tail
                normalize(ctx1, pr, 1, J)
                normalize(ctx0, pr, 0, J)

            def emit_out(J):
                """Output projection for the token blocks of query chunk J."""
                for tb in range(NKB_PER_QC * J, NKB_PER_QC * (J + 1)):
                    o_sb = work2.tile([P, E], f32, tag="o_sb")
                    tsl = slice((tb % NKB_PER_QC) * P, (tb % NKB_PER_QC + 1) * P)
                    for ec in range(NEO):
                        o_ps = ps.tile([P, QC], f32, tag="mm", name="o_ps")
                        for pr in range(NPAIR):
                            nc.tensor.matmul(
                                o_ps[:],
                                ctx_J[J][:, pr, tsl],
                                woT_sb[:, pr, ec * QC : (ec + 1) * QC],
                                start=(pr == 0),
                                stop=(pr == NPAIR - 1),
                            )
                        nc.vector.tensor_copy(
                            o_sb[:, ec * QC : (ec + 1) * QC], o_ps[:]
                        )
                        nc.sync.dma_start(
                            out[tb * P : (tb + 1) * P, ec * QC : (ec + 1) * QC],
                            o_sb[:, ec * QC : (ec + 1) * QC],
                        )

            chain = [(0, 0), (1, 0), (0, 1), (1, 1), (0, 2), (1, 2), (0, 3), (1, 3)]
            pending = []
            chain.pop(0)
            emit_qk_ch(0)
            emit_v(0, NKB_PER_QC)
            emit_attn_pair(0, 0)
            emit_qk_ch(1)
            emit_attn_pair(1, 0)
            emit_out(0)
            emit_v(NKB_PER_QC, 2 * NKB_PER_QC)
            emit_attn_pair(0, 1)
            emit_qk_ch(2)
            emit_attn_pair(1, 1)
            emit_out(1)
            emit_v(2 * NKB_PER_QC, 3 * NKB_PER_QC)
            emit_attn_pair(0, 2)
            emit_qk_ch(3)
            emit_attn_pair(1, 2)
            emit_out(2)
            emit_v(3 * NKB_PER_QC, NTB)
            emit_attn_pair(0, 3)
            emit_attn_pair(1, 3)
            emit_out(3)

    nc.compile()
    return nc


def get_nc():
    global _NC_CACHE
    if _NC_CACHE is None:
        _NC_CACHE = _build_nc()
    return _NC_CACHE


def _round_fp32r(a):
    """Round-to-nearest-even onto the fp32r grid (11 mantissa bits)."""
    b = np.ascontiguousarray(a, dtype=np.float32).view(np.uint32)
    b = b + 0x7FF + ((b >> 12) & 1)
    b &= np.uint32(0xFFFFF000)
    return b.view(np.float32)


def make_in_maps(x, Wq, Wk, Wv, Wo):
    x = np.asarray(x, dtype=np.float32)
    Wq = np.asarray(Wq, dtype=np.float32)
    Wk = np.asarray(Wk, dtype=np.float32)
    Wv = np.asarray(Wv, dtype=np.float32)
    Wo = np.asarray(Wo, dtype=np.float32)
    in_maps = []
    for c in range(N_CORES):
        b, g = divmod(c, TP)
        sl = slice(DL * g, DL * (g + 1))
        in_maps.append(
            {
                "xT": _round_fp32r(x[b].T),
                "wqT": _round_fp32r(Wq[sl].T),
                "wkT": _round_fp32r(Wk[sl].T),
                "wvT": _round_fp32r(Wv[sl].T),
                "woT": _round_fp32r(Wo[:, sl].T),
            }
        )
    return in_maps


def _combine(results, bo):
    bo = np.asarray(bo, dtype=np.float32)
    y = np.zeros((B, S, E), dtype=np.float32)
    for c in range(N_CORES):
        y[c // TP] += results[c]["out"]
    y += bo
    return y


def kernel(x, Wq, Wk, Wv, Wo, bo):
    nc = get_nc()
    in_maps = make_in_maps(x, Wq, Wk, Wv, Wo)
    res = run_bass_kernel_spmd(nc, in_maps, list(range(N_CORES)))
    return _combine(res.results, bo)


def kernel_traced(x, Wq, Wk, Wv, Wo, bo, trace_cores=None):
    """Like kernel() but with NTFF tracing; returns (output, BassKernelResults)."""
    nc = get_nc()
    in_maps = make_in_maps(x, Wq, Wk, Wv, Wo)
    res = run_bass_kernel_spmd(
        nc, in_maps, list(range(N_CORES)), trace=True, trace_cores=trace_cores
    )
    return _combine(res.results, bo), res



# revision 3
# speedup vs baseline: 1.1308x; 1.1308x over previous
"""Trainium2 Bass kernel for nn_MultiHeadAttention_55894704390646.

Multi-head causal attention, B=2, S=2048, E=1024, H=16 heads, D=64.
Sharding: data-parallel over batch (2 groups) x tensor-parallel over heads
(4 heads per core). Each core computes a partial output-projection result
(row-split Wo); the host sums the 4 partials per batch and adds the bias.

v2 layout/schedule (per core):
  - x and Wq/Wk/Wv ship as bf16 (halves the DMA-gated startup); q/k/v and
    the attention probabilities pT are bf16; ctx and the output projection
    stay f32r for accuracy. PSUM accumulation is always fp32.
  - chunks processed in REVERSED order (J=3..0) so the final attention
    chunk is the smallest (4 k-blocks) -> short serial tail.
  - causal column restriction: diagonal k-block di only computes/exps/AVs
    query columns >= 128*di; the remaining 128x128 triangle is masked with
    one shared tri mask (DVE, bf16).
  - output projection for chunk J is LAGGED: its 8 groups are sprinkled
    into the following attention pairs so its dependency latency (normalize
    + r1 DMA shift) hides under other PE work.
  - PSUM evacuations run on DVE; ACT does only the exps; the two heads of a
    pair share one reciprocal tile and one gpsimd partition-broadcast.
"""

import sys

if "/opt/trn_rl_repo" not in sys.path:
    sys.path.insert(0, "/opt/trn_rl_repo")

import numpy as np
import ml_dtypes

import concourse.bass as bass
from concourse import bacc
import concourse.mybir as mybir
import concourse.tile as tile
from concourse.bass_utils import run_bass_kernel_spmd

B, S, E, H, D = 2, 2048, 1024, 16, 64
N_CORES = 8
DP = 2                 # batch groups
TP = 4                 # cores per batch group
HL = H // TP           # local heads per core = 4
DL = HL * D            # local head dims = 256
P = 128
NTB = S // P           # token blocks = 16
QC = 512               # query chunk
NQC = S // QC          # query chunks = 4
NKB = QC // P          # k-blocks per q chunk = 4
NPAIR = HL // 2        # head pairs = 2
NEO = E // QC          # output feature chunks of 512 = 2
NKO = E // P           # contraction blocks over E = 8

f32 = mybir.dt.float32
f32r = mybir.dt.float32r
bf16 = mybir.dt.bfloat16
EXP = mybir.ActivationFunctionType.Exp
MULT = mybir.AluOpType.mult

_NC_CACHE = None


def _build_nc():
    nc = bacc.Bacc("TRN2", target_bir_lowering=False, debug=False)

    xT = nc.dram_tensor("xT", (E, S), bf16, kind="ExternalInput")
    wqT = nc.dram_tensor("wqT", (E, DL), bf16, kind="ExternalInput")
    wkT = nc.dram_tensor("wkT", (E, DL), bf16, kind="ExternalInput")
    wvT = nc.dram_tensor("wvT", (E, DL), bf16, kind="ExternalInput")
    woT = nc.dram_tensor("woT", (DL, E), f32r, kind="ExternalInput")
    out = nc.dram_tensor("out", (S, E), f32, kind="ExternalOutput")

    with tile.TileContext(nc) as tc:
        with (
            nc.allow_low_precision(reason="bf16/f32r are the intended matmul dtypes"),
            tc.tile_pool(name="big", bufs=1) as big,
            tc.tile_pool(name="work", bufs=4) as work,
            tc.tile_pool(name="work2", bufs=2) as work2,
            tc.tile_pool(name="osb", bufs=3) as osb,
            tc.tile_pool(name="ps_mm", bufs=2, space="PSUM") as ps_mm,
            tc.tile_pool(name="ps_s", bufs=2, space="PSUM") as ps_s,
            tc.tile_pool(name="ps_ctx", bufs=2, space="PSUM") as ps_ctx,
        ):
            # ---- DMA loads, ordered so the k-projection of chunk 0 can
            # start as early as possible.
            wkT_sb = big.tile([P, NKO, DL], bf16, tag="wkT")
            nc.sync.dma_start(wkT_sb[:], wkT[:].rearrange("(ko p) d -> p ko d", p=P))
            xT_r = xT[:].rearrange("(ko p) (c s) -> p ko c s", p=P, c=NQC)
            xT_q = [
                [
                    big.tile([P, QC], bf16, tag=f"xTq{ko}_{c}", name=f"xTq{ko}_{c}")
                    for c in range(NQC)
                ]
                for ko in range(NKO)
            ]
            for c in range(NQC):
                for ko in range(NKO):
                    nc.sync.dma_start(xT_q[ko][c][:], xT_r[:, ko, c, :])
                if c == 0:
                    wqT_sb = big.tile([P, NKO, DL], bf16, tag="wqT")
                    nc.sync.dma_start(
                        wqT_sb[:], wqT[:].rearrange("(ko p) d -> p ko d", p=P)
                    )
            wvT_sb = big.tile([P, NKO, DL], bf16, tag="wvT")
            nc.sync.dma_start(wvT_sb[:], wvT[:].rearrange("(ko p) d -> p ko d", p=P))
            woT_sb = big.tile([P, NPAIR, E], f32r, tag="woT")
            nc.sync.dma_start(woT_sb[:], woT[:].rearrange("(pr p) e -> p pr e", p=P))

            # shared causal triangle mask: tri[k, c] = 1 if c >= k else 0
            tri = big.tile([P, P], bf16, tag="tri")
            nc.gpsimd.memset(tri[:], 1.0)
            nc.gpsimd.affine_select(
                out=tri[:],
                in_=tri[:],
                compare_op=mybir.AluOpType.is_ge,
                fill=0.0,
                base=0,
                pattern=[[1, P]],
                channel_multiplier=-1,
            )

            # persistent activation buffers
            qT_c = [[None] * NQC for _ in range(NPAIR)]
            kT_c = [[None] * NQC for _ in range(NPAIR)]
            for pr in range(NPAIR):
                for ch in range(NQC):
                    qT_c[pr][ch] = big.tile(
                        [P, QC], bf16, tag=f"qT{pr}{ch}", name=f"qT{pr}{ch}"
                    )
                    kT_c[pr][ch] = big.tile(
                        [P, QC], bf16, tag=f"kT{pr}{ch}", name=f"kT{pr}{ch}"
                    )
            v_tb = []
            for tb in range(NTB):
                vt = big.tile([P, HL, D + 1], bf16, tag=f"v{tb}", name=f"v{tb}")
                nc.gpsimd.memset(vt[:, :, D], 1.0)
                v_tb.append(vt)
            ctx_J = []
            for J in range(NQC):
                ctx_J.append(
                    big.tile([P, NPAIR, QC], f32r, tag=f"ctxT{J}", name=f"ctxT{J}")
                )

            def emit_kproj(ch):
                for pr in range(NPAIR):
                    pp = ps_mm.tile([P, QC], f32, tag="mm", name=f"ppk_{pr}_{ch}")
                    for ko in range(NKO):
                        nc.tensor.matmul(
                            pp[:],
                            wkT_sb[:, ko, pr * P : (pr + 1) * P],
                            xT_q[ko][ch][:],
                            start=(ko == 0),
                            stop=(ko == NKO - 1),
                        )
                    nc.vector.tensor_copy(kT_c[pr][ch][:], pp[:])

            def emit_qproj(ch):
                for pr in range(NPAIR):
                    pp = ps_mm.tile([P, QC], f32, tag="mm", name=f"ppq_{pr}_{ch}")
                    for ko in range(NKO):
                        nc.tensor.matmul(
                            pp[:],
                            wqT_sb[:, ko, pr * P : (pr + 1) * P],
                            xT_q[ko][ch][:],
                            start=(ko == 0),
                            stop=(ko == NKO - 1),
                        )
                    nc.vector.tensor_copy(qT_c[pr][ch][:], pp[:])

            def emit_v(tb0, tb1):
                for tb in range(tb0, tb1):
                    pv_full = ps_mm.tile([P, QC], f32, tag="mm", name=f"pv{tb}")
                    pv = pv_full[:, 0:DL]
                    for ko in range(NKO):
                        nc.tensor.matmul(
                            pv[:],
                            xT_q[ko][tb // NKB][
                                :, (tb % NKB) * P : (tb % NKB + 1) * P
                            ],
                            wvT_sb[:, ko, :],
                            start=(ko == 0),
                            stop=(ko == NKO - 1),
                        )
                    nc.vector.tensor_copy(
                        v_tb[tb][:, :, 0:D],
                        pv[:].rearrange("p (h d) -> p h d", h=HL),
                    )

            def scores_group(pr, J, I):
                """Scores + exp (+ triangle mask) for k-block I of (pr, J).

                Diagonal blocks (di >= 0) only compute query columns
                >= 128*di; everything left of that is fully causal-masked."""
                di = I - NKB * J
                q0 = P * di if di >= 0 else 0
                kch = I // NKB
                ik = slice((I % NKB) * P, (I % NKB + 1) * P)
                s = ps_s.tile([P, 2, QC], f32, tag="s", name="s")
                nc.tensor.matmul(
                    s[:, 0, q0:QC],
                    kT_c[pr][kch][0:64, ik],
                    qT_c[pr][J][0:64, q0:QC],
                    start=True,
                    stop=True,
                )
                nc.tensor.matmul(
                    s[:, 1, q0:QC],
                    kT_c[pr][kch][64:128, ik],
                    qT_c[pr][J][64:128, q0:QC],
                    start=True,
                    stop=True,
                )
                pT = work.tile([P, 2, QC], bf16, tag="pT", name="pT")
                nc.scalar.activation(pT[:, :, q0:QC], s[:, :, q0:QC], EXP, scale=0.125)
                if di >= 0:
                    nc.vector.tensor_tensor(
                        pT[:, :, q0 : q0 + P],
                        pT[:, :, q0 : q0 + P],
                        tri[:, None, :].to_broadcast((P, 2, P)),
                        MULT,
                    )
                return (pT, q0)

            def normalize(ctx_ps, dnb, pr, r, J):
                """ctxT[h] = ctx[:64] / ctx[64] into its pair slot."""
                if r == 0:
                    nc.vector.tensor_tensor(
                        ctx_J[J][0:64, pr, :],
                        ctx_ps[0:D, :],
                        dnb[:, 0, :],
                        MULT,
                    )
                else:
                    tmp = work2.tile([64, QC], f32r, tag="ctmp")
                    nc.vector.tensor_tensor(
                        tmp[:], ctx_ps[0:D, :], dnb[:, 1, :], MULT
                    )
                    nc.sync.dma_start(ctx_J[J][64:128, pr, :], tmp[:])

            def emit_out_group(J, tb, ec):
                o_ps = ps_mm.tile([P, QC], f32, tag="mm", name=f"o{J}_{tb}_{ec}")
                tsl = slice((tb % NKB) * P, (tb % NKB + 1) * P)
                for pr in range(NPAIR):
                    nc.tensor.matmul(
                        o_ps[:],
                        ctx_J[J][:, pr, tsl],
                        woT_sb[:, pr, ec * QC : (ec + 1) * QC],
                        start=(pr == 0),
                        stop=(pr == NPAIR - 1),
                    )
                o_sb = osb.tile([P, QC], f32, tag="o_sb")
                nc.vector.tensor_copy(o_sb[:], o_ps[:])
                nc.sync.dma_start(
                    out[tb * P : (tb + 1) * P, ec * QC : (ec + 1) * QC],
                    o_sb[:],
                )

            # fillers: lagged output-projection groups, sprinkled into later
            # attention pairs to hide their dependency latency
            filler_q = []

            def enqueue_out(J):
                for tb in range(NKB * J, NKB * (J + 1)):
                    for ec in range(NEO):
                        filler_q.append((J, tb, ec))

            def emit_attn_pair(pr, J):
                """Attention for head pair (2pr, 2pr+1) on query chunk J.
                AVs are skewed one k-block behind the scores; the next
                pair's first scores group is prefetched before the last AV
                so the PE queue never drains at pair boundaries."""
                h0, h1 = 2 * pr, 2 * pr + 1
                nI = NKB * (J + 1)
                ctx0 = ps_ctx.tile([D + 1, QC], f32, tag="ctx", name="ctx0")
                ctx1 = ps_ctx.tile([D + 1, QC], f32, tag="ctx", name="ctx1")

                def emit_av(I, pTq):
                    pT, q0 = pTq
                    nc.tensor.matmul(
                        ctx0[:, q0:QC], v_tb[I][:, h0, :], pT[:, 0, q0:QC],
                        start=(I == 0), stop=(I == nI - 1),
                        skip_group_check=True,
                    )
                    nc.tensor.matmul(
                        ctx1[:, q0:QC], v_tb[I][:, h1, :], pT[:, 1, q0:QC],
                        start=(I == 0), stop=(I == nI - 1),
                        skip_group_check=True,
                    )

                prev_pT = pending.pop() if pending else scores_group(pr, J, 0)
                for I in range(1, nI):
                    pT = scores_group(pr, J, I)
                    emit_av(I - 1, prev_pT)
                    prev_pT = pT
                    if filler_q and I % 2 == 0 and I >= 4:
                        emit_out_group(*filler_q.pop(0))
                nxt = chain.pop(0) if chain else None
                if nxt is not None:
                    pending.append(scores_group(nxt[0], nxt[1], 0))
                emit_av(nI - 1, prev_pT)
                # shared reciprocal + broadcast for both heads of the pair;
                # r=1 first so its DMA shift latency stays off the tail.
                # (reciprocal_approx_fast is a bitwise custom DVE op and
                # reads garbage from PSUM -> stage the den rows via ACT.)
                den_sb = work2.tile([1, 2, QC], f32, tag="den", name="den")
                nc.scalar.copy(den_sb[:, 1, :], ctx1[D : D + 1, :])
                nc.scalar.copy(den_sb[:, 0, :], ctx0[D : D + 1, :])
                rec = work2.tile([1, 2, QC], f32, tag="rec", name="rec")
                nc.vector.reciprocal_approx_fast(rec[:], den_sb[:])
                dnb = work2.tile([64, 2, QC], f32, tag="dnb", name="dnb")
                nc.gpsimd.partition_broadcast(dnb[:], rec[:])
                normalize(ctx1, dnb, pr, 1, J)
                normalize(ctx0, dnb, pr, 0, J)

            # ---- schedule: reversed chunk order
            chain = [(0, 3), (1, 3), (0, 2), (1, 2), (0, 1), (1, 1), (0, 0), (1, 0)]
            pending = []
            for ch in range(NQC):
                emit_kproj(ch)
            emit_qproj(3)
            emit_v(0, NTB)
            chain.pop(0)
            emit_attn_pair(0, 3)
            emit_qproj(2)
            emit_attn_pair(1, 3)
            enqueue_out(3)
            emit_attn_pair(0, 2)
            emit_qproj(1)
            emit_attn_pair(1, 2)
            enqueue_out(2)
            emit_attn_pair(0, 1)
            emit_qproj(0)
            emit_attn_pair(1, 1)
            enqueue_out(1)
            emit_attn_pair(0, 0)
            emit_attn_pair(1, 0)
            enqueue_out(0)
            while filler_q:
                emit_out_group(*filler_q.pop(0))

    nc.compile()
    return nc


def get_nc():
    global _NC_CACHE
    if _NC_CACHE is None:
        _NC_CACHE = _build_nc()
    return _NC_CACHE


def _round_fp32r(a):
    """Round-to-nearest-even onto the fp32r grid (11 mantissa bits)."""
    b = np.ascontiguousarray(a, dtype=np.float32).view(np.uint32)
    b = b + 0x7FF + ((b >> 12) & 1)
    b &= np.uint32(0xFFFFF000)
    return b.view(np.float32)


def _bf16(a):
    return np.ascontiguousarray(a, dtype=np.float32).astype(ml_dtypes.bfloat16)


def make_in_maps(x, Wq, Wk, Wv, Wo):
    x = np.asarray(x, dtype=np.float32)
    Wq = np.asarray(Wq, dtype=np.float32)
    Wk = np.asarray(Wk, dtype=np.float32)
    Wv = np.asarray(Wv, dtype=np.float32)
    Wo = np.asarray(Wo, dtype=np.float32)
    in_maps = []
    for c in range(N_CORES):
        b, g = divmod(c, TP)
        sl = slice(DL * g, DL * (g + 1))
        in_maps.append(
            {
                "xT": _bf16(x[b].T),
                "wqT": _bf16(Wq[sl].T),
                "wkT": _bf16(Wk[sl].T),
                "wvT": _bf16(Wv[sl].T),
                "woT": _round_fp32r(Wo[:, sl].T),
            }
        )
    return in_maps


def _combine(results, bo):
    bo = np.asarray(bo, dtype=np.float32)
    y = np.zeros((B, S, E), dtype=np.float32)
    for c in range(N_CORES):
        y[c // TP] += results[c]["out"]
    y += bo
    return y


def kernel(x, Wq, Wk, Wv, Wo, bo):
    nc = get_nc()
    in_maps = make_in_maps(x, Wq, Wk, Wv, Wo)
    res = run_bass_kernel_spmd(nc, in_maps, list(range(N_CORES)))
    return _combine(res.results, bo)


def kernel_traced(x, Wq, Wk, Wv, Wo, bo, trace_cores=None):
    """Like kernel() but with NTFF tracing; returns (output, BassKernelResults)."""
    nc = get_nc()
    in_maps = make_in_maps(x, Wq, Wk, Wv, Wo)
    res = run_bass_kernel_spmd(
        nc, in_maps, list(range(N_CORES)), trace=True, trace_cores=trace_cores
    )
    return _combine(res.results, bo), res
